# revision 19
# baseline (speedup 1.0000x reference)
"""Causal self-attention with anchor-relative rope (ferope), 8-core TRN2 Bass kernel.

Full-scale problem: B=2, T=2048, C=2048, H=16, D=128, M=32.

Sharding (tensor-parallel heads + data-parallel batch):
  - 8 cores = 2 batch groups x 4 cores. Core (g, pos) handles batch g, local
    heads 0..3 = global heads pos*4..pos*4+3.
  - All weights/x are cast to bf16 on the HOST and DMA'd directly (no on-device
    staging/casting). Rope sin/cos tables and causal masks are also
    host-precomputed (bf16) so the device setup phase is pure DMA.
  - qkv: per panel of 512 timesteps, kb-outer accumulation so matmuls start as
    soon as the first DMA chunks land. Rope is fused per panel on the DVE in
    bf16 right after each q/k psum->sbuf copy.
  - attention uses transposed scores s_T[ki,qi]; causal structure exploited at
    128-column granularity on diagonal blocks (sub-window matmuls/exp).
  - rowsum of exp via ones-stationary matmul accumulated in PSUM.
  - y slices AllGathered per head within each 4-core batch group (Shared-output
    collectives); head 3 is gathered in two T-halves so the output projection
    tail overlaps the last gather.
  - output projection is column-sharded; accumulated over head-chunks in SBUF
    f32 so each chunk only needs its own head's gather.
"""

import math

import numpy as np
import ml_dtypes

import concourse.bass as bass
import concourse.mybir as mybir
import concourse.tile as tile
from concourse import bacc
from concourse.bass_utils import run_bass_kernel_spmd

F32 = mybir.dt.float32
BF16 = mybir.dt.bfloat16

# full-scale dims (hardcoded per harness contract)
B, T, C, H, DH, M = 2, 2048, 2048, 16, 128, 32
N_CORES = 8
GROUPS = 2                     # batch groups
CPG = N_CORES // GROUPS        # cores per group = 4
HPC = H // CPG                 # heads per core = 4
C_LOC = HPC * DH               # 512: per-core head channels
PANEL = 512                    # qi panel width (one psum bank)
KB = 128                       # ki block (partition dim)
N_CB = C // KB                 # 16 contraction blocks
N_TB = T // KB                 # 16 timestep blocks
N_PANELS = T // PANEL          # 4
KB_PER_PANEL = PANEL // KB     # 4


def build_program():
    """Build the SPMD Bass program (same program on all cores; data differs)."""
    inv_sqrt_d = 1.0 / math.sqrt(DH)

    nc = bacc.Bacc("TRN2", target_bir_lowering=False, debug=False,
                   num_devices=N_CORES)

    # all tensors host-pre-tiled to [partition, kb, cols] so every DMA reads
    # long contiguous per-partition lines (full HBM bandwidth)
    xp_d = nc.dram_tensor("xp", [N_PANELS, KB, N_CB, PANEL], BF16,
                          kind="ExternalInput").ap()
    wqk_d = nc.dram_tensor("wqk", [KB, N_CB, 2 * C_LOC], BF16,
                           kind="ExternalInput").ap()
    wv_d = nc.dram_tensor("wv", [KB, N_CB, C_LOC], BF16,
                          kind="ExternalInput").ap()
    wo_d = nc.dram_tensor("wo", [KB, N_CB, C_LOC], BF16,
                          kind="ExternalInput").ap()
    tab_d = nc.dram_tensor("tab", [2 * M, 2, T], BF16, kind="ExternalInput").ap()
    masks_d = nc.dram_tensor("masks", [KB, KB_PER_PANEL, PANEL], BF16,
                             kind="ExternalInput").ap()
    out_d = nc.dram_tensor("out", [T, C_LOC], F32, kind="ExternalOutput").ap()

    replica_groups = [list(range(g * CPG, (g + 1) * CPG)) for g in range(GROUPS)]

    with tile.TileContext(nc) as tc:
        with (
            tc.tile_pool(name="dram", bufs=1, space="DRAM") as dram,
            tc.tile_pool(name="const", bufs=1) as const,
            tc.tile_pool(name="qkv", bufs=1) as qkv,
            tc.tile_pool(name="work", bufs=1) as work,
        ):
            # DRAM comm buffers, one per (head, T-half): fine-grained gathers
            # start earlier and each projection chunk depends on just one.
            y_parts = [[dram.tile([KB, T // 2], BF16, name=f"yp{h}_{hf}")
                        for hf in range(2)] for h in range(HPC)]
            y_alls = [[dram.tile([CPG * KB, T // 2], BF16, name=f"ya{h}_{hf}")
                       for hf in range(2)] for h in range(HPC)]

            # weights (host-cast bf16), chunked DMAs for fine-grained deps
            wv_sb = const.tile([KB, N_CB, C_LOC], BF16)
            for ci in range(2):
                nc.sync.dma_start(out=wv_sb[:, 8 * ci:8 * ci + 8, :],
                                  in_=wv_d[:, 8 * ci:8 * ci + 8, :])
            wqk_sb = const.tile([KB, N_CB, 2 * C_LOC], BF16)
            for ci in range(8):
                nc.sync.dma_start(out=wqk_sb[:, 2 * ci:2 * ci + 2, :],
                                  in_=wqk_d[:, 2 * ci:2 * ci + 2, :])
            # tab[:, 0, :] = [-sin; +sin], tab[:, 1, :] = [cos; cos]
            tab_sb = const.tile([2 * M, 2, T], BF16)
            nc.sync.dma_start(out=tab_sb[:], in_=tab_d)
            ones128 = const.tile([KB, KB], BF16)
            nc.vector.memset(ones128[:], 1.0)
            masks_sb = const.tile([KB, KB_PER_PANEL, PANEL], BF16)
            nc.sync.dma_start(out=masks_sb[:], in_=masks_d)

            # ---- qkv projection + fused rope ----
            # q/k stored per head as [d, t] bf16; v natural [t, d] bf16.
            q_sb = [qkv.tile([DH, T], BF16, name=f"q{h}") for h in range(HPC)]
            k_sb = [qkv.tile([DH, T], BF16, name=f"k{h}") for h in range(HPC)]
            v_all = qkv.tile([KB, N_TB, C_LOC], BF16)
            # cb -> destination tile: [q0, k0, q1, k1, q2, k2, q3, k3]
            qk_dst = [t for pair in zip(q_sb, k_sb) for t in pair]

            def rope(dst, tps):
                """In-place ferope on rows 0:2M of dst[:, tps:tps+PANEL]."""
                s = dst[0:2 * M, tps:tps + PANEL]
                sw = work.tile([2 * M, PANEL], BF16, tag="sw", bufs=3)
                nc.vector.tensor_copy(sw[0:M, :], dst[M:2 * M, tps:tps + PANEL])
                nc.vector.tensor_copy(sw[M:2 * M, :], dst[0:M, tps:tps + PANEL])
                nc.vector.tensor_mul(sw[:], sw[:], tab_sb[:, 0, tps:tps + PANEL])
                nc.vector.tensor_mul(s, s, tab_sb[:, 1, tps:tps + PANEL])
                nc.vector.tensor_add(s, s, sw[:])

            # warmup collective: absorbs the one-time ring-setup cost of the
            # first collective (~40us) while qkv matmuls run. Dummy data.
            warm_in = dram.tile([KB, 64], BF16)
            warm_out = dram.tile([CPG * KB, 64], BF16)

            with tc.tile_pool(name="xpool", bufs=1) as xpool:
                xbs = []
                for tp in range(N_PANELS):
                    xb = xpool.tile([KB, N_CB, PANEL], BF16, tag="xb", bufs=2,
                                    name=f"xb{tp}")
                    if tp == 0:
                        # fine chunks so the first matmuls start early
                        for ci in range(2):
                            nc.gpsimd.dma_start(
                                out=xb[:, 8 * ci:8 * ci + 8, :],
                                in_=xp_d[tp, :, 8 * ci:8 * ci + 8, :])
                    elif tp == 1:
                        nc.gpsimd.dma_start(out=xb[:], in_=xp_d[tp])
                        # warmup here: the CC firmware takes ~65us after NEFF
                        # start before it can serve any collective, and the
                        # gpsimd queue blocks while a collective runs — so it
                        # must sit after the xb1 issue but before xb2/xb3
                        # (whose buffer-reuse waits would delay it further).
                        nc.gpsimd.collective_compute(
                            "AllGather", mybir.AluOpType.bypass,
                            replica_groups=replica_groups,
                            ins=[warm_in[:]], outs=[warm_out[:]])
                    else:
                        nc.gpsimd.dma_start(out=xb[:], in_=xp_d[tp])
                    xbs.append(xb)

                with tc.tile_pool(name="psq", bufs=1, space="PSUM") as psq:
                    for tp in range(N_PANELS):
                        xb = xbs[tp]
                        tps = tp * PANEL
                        # pass A: v blocks, kb-outer
                        pvs = [psq.tile([KB, C_LOC], F32, tag="pv", bufs=4,
                                        name=f"pv{tp}_{t}") for t in range(4)]
                        for kb in range(N_CB):
                            for tbl in range(KB_PER_PANEL):
                                nc.tensor.matmul(
                                    pvs[tbl][:],
                                    xb[:, kb, tbl * KB:(tbl + 1) * KB],
                                    wv_sb[:, kb, :],
                                    start=(kb == 0), stop=(kb == N_CB - 1))
                        for tbl in range(KB_PER_PANEL):
                            nc.scalar.copy(v_all[:, tp * KB_PER_PANEL + tbl, :],
                                           pvs[tbl][:])
                        # pass B: q/k column blocks in two halves, kb-outer
                        for half in range(2):
                            pqks = [psq.tile([DH, PANEL], F32, tag="pqk",
                                             bufs=4, name=f"pqk{tp}_{half}_{j}")
                                    for j in range(4)]
                            for kb in range(N_CB):
                                for j in range(4):
                                    cb = half * 4 + j
                                    nc.tensor.matmul(
                                        pqks[j][:],
                                        wqk_sb[:, kb, cb * DH:(cb + 1) * DH],
                                        xb[:, kb, :],
                                        start=(kb == 0), stop=(kb == N_CB - 1))
                            for j in range(4):
                                cb = half * 4 + j
                                dst = qk_dst[cb]
                                nc.scalar.copy(dst[:, tps:tps + PANEL],
                                               pqks[j][:])
                                rope(dst, tps)

            # ---- causal attention per head + per-head AllGather ----
            with tc.tile_pool(name="proj", bufs=1) as proj:
                # prefetch proj weights during attention
                wo_sb = proj.tile([KB, N_CB, C_LOC], BF16)
                for ci in range(2):
                    nc.sync.dma_start(out=wo_sb[:, 8 * ci:8 * ci + 8, :],
                                      in_=wo_d[:, 8 * ci:8 * ci + 8, :])

                psa_cm = tc.tile_pool(name="psa", bufs=1, space="PSUM")
                psa = psa_cm.__enter__()
                pso_cm = tc.tile_pool(name="pso", bufs=1, space="PSUM")
                pso = pso_cm.__enter__()
                out_acc = [proj.tile([KB, C_LOC], F32, name=f"oacc{i}")
                           for i in range(N_TB)]

                for h in range(HPC):
                    qh, kh = q_sb[h], k_sb[h]
                    for J in range(N_PANELS):
                        nkb = (J + 1) * KB_PER_PANEL
                        qs = J * PANEL
                        py = psa.tile([DH, PANEL], F32, tag="y", bufs=2)
                        pr = psa.tile([KB, PANEL], F32, tag="r", bufs=2)
                        for b in range(nkb):
                            p = b - KB_PER_PANEL * J
                            o = KB * p if p > 0 else 0  # causal col window
                            ps = psa.tile([KB, PANEL], F32, tag="s", bufs=2)
                            nc.tensor.matmul(
                                ps[:, o:PANEL],
                                kh[:, b * KB:(b + 1) * KB],
                                qh[:, qs + o:qs + PANEL],
                                start=True, stop=True)
                            et = work.tile([KB, PANEL], BF16, tag="exp",
                                           bufs=4)
                            nc.scalar.activation(
                                et[:, o:PANEL], ps[:, o:PANEL],
                                mybir.ActivationFunctionType.Exp,
                                scale=inv_sqrt_d)
                            if p >= 0:
                                nc.vector.tensor_mul(
                                    et[:, o:PANEL], et[:, o:PANEL],
                                    masks_sb[:, p, o:PANEL])
                            nc.tensor.matmul(
                                py[:, o:PANEL],
                                v_all[:, b, h * DH:(h + 1) * DH],
                                et[:, o:PANEL],
                                start=(b == 0), stop=(b == nkb - 1))
                            nc.tensor.matmul(
                                pr[:, o:PANEL], ones128[:], et[:, o:PANEL],
                                start=(b == 0), stop=(b == nkb - 1))
                        # normalize: y * (1/rowsum)
                        rinv = work.tile([KB, PANEL], F32, tag="rinv", bufs=2)
                        nc.vector.reciprocal_approx_fast(rinv[:], pr[:])
                        ysb = work.tile([DH, PANEL], BF16, tag="ysb", bufs=3)
                        nc.vector.tensor_mul(ysb[:], py[:], rinv[:])
                        hf = J // 2
                        cs = qs - hf * (T // 2)
                        nc.sync.dma_start(
                            out=y_parts[h][hf][:, cs:cs + PANEL], in_=ysb[:])
                        if J % 2 == 1:
                            nc.gpsimd.collective_compute(
                                "AllGather", mybir.AluOpType.bypass,
                                replica_groups=replica_groups,
                                ins=[y_parts[h][hf][:]],
                                outs=[y_alls[h][hf][:]])

                # ---- output projection, (head-chunk, T-half)-major so each
                # chunk only depends on one fine-grained gather
                y_all_ts = [[y_alls[h][hf][:].rearrange("(g p) t -> p g t",
                                                        g=CPG)
                             for hf in range(2)] for h in range(HPC)]
                for hf in range(2):
                    for hh in range(HPC):
                        for ld in range(4):   # 4 loads of 2 tb each
                            yt = work.tile([KB, CPG, 2 * KB], BF16, tag="yt",
                                           bufs=6)
                            src = y_all_ts[hh][hf][:, :,
                                                   2 * ld * KB:
                                                   (2 * ld + 2) * KB]
                            nc.sync.dma_start(out=yt[:], in_=src)
                            for sub in range(2):
                                tb = hf * (N_TB // 2) + ld * 2 + sub
                                po = pso.tile([KB, C_LOC], F32, tag="po",
                                              bufs=2)
                                for g in range(CPG):
                                    nc.tensor.matmul(
                                        po[:],
                                        yt[:, g, sub * KB:(sub + 1) * KB],
                                        wo_sb[:, g * HPC + hh, :],
                                        start=(g == 0), stop=(g == CPG - 1))
                                if hh == 0:
                                    nc.vector.tensor_copy(out_acc[tb][:],
                                                          po[:])
                                else:
                                    nc.vector.tensor_add(out_acc[tb][:],
                                                         out_acc[tb][:],
                                                         po[:])
                                if hh == HPC - 1:
                                    nc.scalar.dma_start(
                                        out=out_d[tb * KB:(tb + 1) * KB, :],
                                        in_=out_acc[tb][:])

                pso_cm.__exit__(None, None, None)
                psa_cm.__exit__(None, None, None)

    nc.compile()
    return nc


def make_in_maps(x, w_attn, w_proj, freqs, delta):
    """Host-side sharding: slice/transpose/cast full inputs into per-core maps."""
    bf16 = ml_dtypes.bfloat16
    x = np.asarray(x, dtype=np.float32)
    w_attn = np.asarray(w_attn, dtype=np.float32)
    w_proj = np.asarray(w_proj, dtype=np.float32)
    freqs = np.asarray(freqs, dtype=np.float32)
    delta = np.asarray(delta, dtype=np.float32)

    # rope tables: tab[0:M,0] = -sin, tab[M:2M,0] = +sin, tab[:,1] = cos
    ang = delta[:, None].astype(np.float64) * freqs[None, :].astype(np.float64)
    sin_t = np.sin(ang).T.astype(np.float32)   # [M, T]
    cos_t = np.cos(ang).T.astype(np.float32)
    tab = np.empty((2 * M, 2, T), np.float32)
    tab[0:M, 0] = -sin_t
    tab[M:2 * M, 0] = sin_t
    tab[0:M, 1] = cos_t
    tab[M:2 * M, 1] = cos_t
    tab = tab.astype(bf16)

    # causal masks: masks[ki, p, qi] = 1 if qi >= ki + 128*p
    ki = np.arange(KB)[:, None, None]
    pp = np.arange(KB_PER_PANEL)[None, :, None]
    qi = np.arange(PANEL)[None, None, :]
    masks = (qi >= ki + KB * pp).astype(bf16)

    def ptile(w):
        """[C, cols] -> [p, kb, cols] partition-tiled contiguous layout."""
        return np.ascontiguousarray(
            w.reshape(N_CB, KB, -1).transpose(1, 0, 2).astype(bf16))

    in_maps = []
    for core in range(N_CORES):
        g, pos = divmod(core, CPG)
        heads = range(pos * HPC, (pos + 1) * HPC)
        # [C, T] -> [panel, p, kb, t] pre-tiled contiguous layout
        xp = np.ascontiguousarray(
            x[g].T.reshape(N_CB, KB, N_PANELS, PANEL)
            .transpose(2, 1, 0, 3).astype(bf16))
        # cb order: q0, k0, q1, k1, q2, k2, q3, k3 (local heads)
        wqk_cols = []
        for h in heads:
            wqk_cols.append(w_attn[:, h * DH:(h + 1) * DH])
            wqk_cols.append(w_attn[:, C + h * DH:C + (h + 1) * DH])
        wqk = ptile(np.concatenate(wqk_cols, axis=1))
        wv = ptile(np.concatenate(
            [w_attn[:, 2 * C + h * DH:2 * C + (h + 1) * DH] for h in heads],
            axis=1))
        wo = ptile(w_proj[:, pos * C_LOC:(pos + 1) * C_LOC])
        in_maps.append({
            "xp": xp,
            "wqk": wqk,
            "wv": wv,
            "wo": wo,
            "tab": tab,
            "masks": masks,
        })
    return in_maps


def assemble_output(results):
    outs = []
    for g in range(GROUPS):
        cols = [results[g * CPG + pos]["out"] for pos in range(CPG)]
        outs.append(np.concatenate(cols, axis=1))
    return np.stack(outs, axis=0).astype(np.float32)


_NC_CACHE = {}


def _get_program():
    if "nc" not in _NC_CACHE:
        _NC_CACHE["nc"] = build_program()
    return _NC_CACHE["nc"]


def kernel(x, w_attn, w_proj, freqs, delta):
    nc = _get_program()
    in_maps = make_in_maps(x, w_attn, w_proj, freqs, delta)
    res = run_bass_kernel_spmd(nc, in_maps, list(range(N_CORES)))
    return assemble_output(res.results)
